# revision 4
# baseline (speedup 1.0000x reference)
"""Trainium2 Bass kernel for 4-layer cross-stencil CNN.

Per-core: one image [6,256,256] (batch dim sharded across 8 cores).
conv(cross-5-stencil) = 5 channel-matmuls with spatially shifted rhs APs,
accumulated in PSUM. Channels on partitions, spatial (rows x cols) on the
free dim. fp32r matmuls (full PE rate at N>=256).

Strips of R=32 output rows with overlap-compute for the halos; all four
layers fused in SBUF (no DRAM intermediates).
"""

import sys

sys.path.insert(0, "/opt/trn_rl_repo")

import numpy as np

import concourse.bacc as bacc
import concourse.mybir as mybir
from concourse.tile import TileContext
from concourse import bass_utils

IN_C, HID_C, OUT_C = 6, 128, 6
B, H, W = 8, 256, 256
WP = W + 2  # padded width
R = 32  # output rows per strip
N_CORES = 8

f32 = mybir.dt.float32
f32r = mybir.dt.float32r
Relu = mybir.ActivationFunctionType.Relu
Ident = mybir.ActivationFunctionType.Identity

# tap order matches reference: 0=center, 1=up(x[h-1]), 2=down(x[h+1]),
# 3=left(x[w-1]), 4=right(x[w+1])


def _build():
    nc = bacc.Bacc("TRN2", target_bir_lowering=False)

    x_d = nc.dram_tensor("x", [IN_C, H, W], f32, kind="ExternalInput")
    w1_d = nc.dram_tensor("w1p", [5 * IN_C, HID_C], f32, kind="ExternalInput")
    w2_d = nc.dram_tensor("w2p", [HID_C, 5, HID_C], f32, kind="ExternalInput")
    w3_d = nc.dram_tensor("w3p", [HID_C, 5, HID_C], f32, kind="ExternalInput")
    w4_d = nc.dram_tensor("w4p", [HID_C, 5, OUT_C], f32, kind="ExternalInput")
    b1_d = nc.dram_tensor("b1", [HID_C], f32, kind="ExternalInput")
    b2_d = nc.dram_tensor("b2", [HID_C], f32, kind="ExternalInput")
    b3_d = nc.dram_tensor("b3", [HID_C], f32, kind="ExternalInput")
    b4_d = nc.dram_tensor("b4", [OUT_C], f32, kind="ExternalInput")
    y_d = nc.dram_tensor("y", [OUT_C, H, W], f32, kind="ExternalOutput")

    with TileContext(nc) as tc:
        with (
            tc.tile_pool(name="const", bufs=1) as cpool,
            tc.tile_pool(name="bufs", bufs=1) as bpool,
            tc.tile_pool(name="io", bufs=4) as iopool,
            tc.tile_pool(name="psmain", bufs=6, space="PSUM") as pmain,
            tc.tile_pool(name="ps4", bufs=2, space="PSUM") as p4,
        ):
            # --- weights / biases (resident) ---
            w1_sb = cpool.tile([5 * IN_C, HID_C], f32r)
            nc.sync.dma_start(out=w1_sb, in_=w1_d[:, :].bitcast(f32r))
            w2_sb = cpool.tile([HID_C, 5, HID_C], f32r)
            nc.sync.dma_start(out=w2_sb, in_=w2_d[:, :, :].bitcast(f32r))
            w3_sb = cpool.tile([HID_C, 5, HID_C], f32r)
            nc.sync.dma_start(out=w3_sb, in_=w3_d[:, :, :].bitcast(f32r))
            w4_sb = cpool.tile([HID_C, 5, OUT_C], f32r)
            nc.sync.dma_start(out=w4_sb, in_=w4_d[:, :, :].bitcast(f32r))
            b1_sb = cpool.tile([HID_C, 1], f32)
            nc.sync.dma_start(out=b1_sb, in_=b1_d[:, None])
            b2_sb = cpool.tile([HID_C, 1], f32)
            nc.sync.dma_start(out=b2_sb, in_=b2_d[:, None])
            b3_sb = cpool.tile([HID_C, 1], f32)
            nc.sync.dma_start(out=b3_sb, in_=b3_d[:, None])
            b4_sb = cpool.tile([OUT_C, 1], f32)
            nc.sync.dma_start(out=b4_sb, in_=b4_d[:, None])

            # --- persistent strip buffers (bufs=1; zero-initialized once) ---
            # x30: 5 tap-groups x 6ch, pre-shifted by DMA placement.
            # group g partitions [6g,6g+6); center x(h,w) -> (slot h-a+5, col w+1)
            x30 = bpool.tile([5 * IN_C, R + 10, WP], f32r)
            h1 = bpool.tile([HID_C, R + 6, WP], f32r)  # L1 rows [a-3,b+3)
            h2 = bpool.tile([HID_C, R + 4, WP], f32r)  # L2 rows [a-2,b+2)
            h3 = bpool.tile([HID_C, R + 2, WP], f32r)  # L3 rows [a-1,b+1)

            nc.vector.memset(x30[:, :, :].bitcast(f32), 0.0)
            nc.vector.memset(h1[:, :, :].bitcast(f32), 0.0)
            nc.vector.memset(h2[:, :, :].bitcast(f32), 0.0)
            nc.vector.memset(h3[:, :, :].bitcast(f32), 0.0)

            def conv_chunk(ps, w_sb, src, s, n, first_s, last_s):
                """5 accumulating matmuls: output chunk of n rows whose
                center is src[:, s:s+n, 1:257]. first_s/last_s clamp guards
                are handled by the caller via buffer zero-slots."""
                nc.tensor.matmul(
                    ps, w_sb[:, 0, :], src[:, s : s + n, 1 : 1 + W],
                    start=True, stop=False,
                )
                nc.tensor.matmul(
                    ps, w_sb[:, 1, :], src[:, s - 1 : s - 1 + n, 1 : 1 + W],
                    start=False, stop=False,
                )
                nc.tensor.matmul(
                    ps, w_sb[:, 2, :], src[:, s + 1 : s + 1 + n, 1 : 1 + W],
                    start=False, stop=False,
                )
                nc.tensor.matmul(
                    ps, w_sb[:, 3, :], src[:, s : s + n, 0:W],
                    start=False, stop=False,
                )
                nc.tensor.matmul(
                    ps, w_sb[:, 4, :], src[:, s : s + n, 2 : 2 + W],
                    start=False, stop=True,
                )

            for a in range(0, H, R):
                b = a + R
                last = b == H
                lo_x, hi_x = max(0, a - 4), min(H, b + 4)

                if last:
                    # re-zero stale tail slots (bufs=1 reuse) before writes
                    nc.vector.memset(x30[:, hi_x - a + 4 : R + 10, :].bitcast(f32), 0.0)
                    nc.vector.memset(h1[:, 256 - (a - 3) : R + 6, :].bitcast(f32), 0.0)
                    nc.vector.memset(h2[:, 256 - (a - 2) : R + 4, :].bitcast(f32), 0.0)
                    nc.vector.memset(h3[:, 256 - (a - 1) : R + 2, :].bitcast(f32), 0.0)

                # --- load x strip, 5 shifted placements ---
                src = x_d[:, lo_x:hi_x, :].bitcast(f32r)
                o = lo_x - a
                nc.sync.dma_start(out=x30[0:6, o + 5 : hi_x - a + 5, 1 : 1 + W], in_=src)
                nc.sync.dma_start(out=x30[6:12, o + 6 : hi_x - a + 6, 1 : 1 + W], in_=src)
                nc.sync.dma_start(out=x30[12:18, o + 4 : hi_x - a + 4, 1 : 1 + W], in_=src)
                nc.sync.dma_start(out=x30[18:24, o + 5 : hi_x - a + 5, 2 : 2 + W], in_=src)
                nc.sync.dma_start(out=x30[24:30, o + 5 : hi_x - a + 5, 0:W], in_=src)

                # --- L1: rows [a-3, b+3) -> h1 (single K=30 matmul/chunk) ---
                rr = max(0, a - 3)
                hi = min(H, b + 3)
                while rr < hi:
                    n = min(2, hi - rr)
                    s = rr - a + 5
                    ps = pmain.tile([HID_C, n, W], f32, tag="ps")
                    nc.tensor.matmul(
                        ps, w1_sb[:, :], x30[:, s : s + n, 1 : 1 + W],
                        start=True, stop=True,
                    )
                    d = rr - (a - 3)
                    nc.scalar.activation(
                        h1[:, d : d + n, 1 : 1 + W], ps, Relu, bias=b1_sb
                    )
                    rr += n

                # --- L2: rows [a-2, b+2), reads h1 ---
                rr = max(0, a - 2)
                hi = min(H, b + 2)
                while rr < hi:
                    n = min(2, hi - rr)
                    s = rr - a + 3  # h1 slot of center
                    ps = pmain.tile([HID_C, n, W], f32, tag="ps")
                    conv_chunk(ps, w2_sb, h1, s, n, None, None)
                    d = rr - (a - 2)
                    nc.scalar.activation(
                        h2[:, d : d + n, 1 : 1 + W], ps, Relu, bias=b2_sb
                    )
                    rr += n

                # --- L3: rows [a-1, b+1), reads h2 ---
                rr = max(0, a - 1)
                hi = min(H, b + 1)
                while rr < hi:
                    n = min(2, hi - rr)
                    s = rr - a + 2  # h2 slot of center
                    ps = pmain.tile([HID_C, n, W], f32, tag="ps")
                    conv_chunk(ps, w3_sb, h2, s, n, None, None)
                    d = rr - (a - 1)
                    nc.scalar.activation(
                        h3[:, d : d + n, 1 : 1 + W], ps, Relu, bias=b3_sb
                    )
                    rr += n

                # --- L4: rows [a, b), reads h3, no relu ---
                rr = a
                while rr < b:
                    n = min(2, b - rr)
                    s = rr - a + 1  # h3 slot of center
                    ps = p4.tile([OUT_C, n, W], f32, tag="ps4")
                    conv_chunk(ps, w4_sb, h3, s, n, None, None)
                    yt = iopool.tile([OUT_C, n, W], f32, tag="yt")
                    nc.scalar.activation(yt, ps, Ident, bias=b4_sb)
                    nc.sync.dma_start(out=y_d[:, rr : rr + n, :], in_=yt)
                    rr += n

    nc.finalize()
    return nc


_NC_CACHE = {}


def kernel(x, w1, b1, w2, b2, w3, b3, w4, b4):
    x = np.ascontiguousarray(np.asarray(x, dtype=np.float32))
    w1 = np.asarray(w1, dtype=np.float32)
    w2 = np.asarray(w2, dtype=np.float32)
    w3 = np.asarray(w3, dtype=np.float32)
    w4 = np.asarray(w4, dtype=np.float32)

    # pack weights: lhsT layouts
    # w1p[t*6+ic, oc] = w1[oc, ic, t]
    w1p = np.ascontiguousarray(w1.transpose(2, 1, 0).reshape(5 * IN_C, HID_C))
    # w2p[ic, t, oc] = w2[oc, ic, t]
    w2p = np.ascontiguousarray(w2.transpose(1, 2, 0))
    w3p = np.ascontiguousarray(w3.transpose(1, 2, 0))
    w4p = np.ascontiguousarray(w4.transpose(1, 2, 0))

    if "nc" not in _NC_CACHE:
        _NC_CACHE["nc"] = _build()
    nc = _NC_CACHE["nc"]

    common = {
        "w1p": w1p, "w2p": w2p, "w3p": w3p, "w4p": w4p,
        "b1": np.asarray(b1, np.float32), "b2": np.asarray(b2, np.float32),
        "b3": np.asarray(b3, np.float32), "b4": np.asarray(b4, np.float32),
    }
    in_maps = [dict(common, x=x[i]) for i in range(N_CORES)]
    res = bass_utils.run_bass_kernel_spmd(nc, in_maps, core_ids=list(range(N_CORES)))
    out = np.stack([res.results[i]["y"] for i in range(N_CORES)], axis=0)
    return out


# revision 5
# speedup vs baseline: 10.4203x; 10.4203x over previous
"""Trainium2 Bass kernel for 4-layer cross-stencil CNN.

Per-core: one image [6,256,256] (batch dim sharded across 8 cores).
conv(cross-5-stencil) = 5 channel-matmuls with spatially shifted rhs APs,
accumulated in PSUM. Channels on partitions, spatial (rows x cols) on the
free dim. fp32r matmuls (full PE rate at N>=256).

Strips of R=32 output rows with overlap-compute for the halos; all four
layers fused in SBUF (no DRAM intermediates).
"""

import sys

sys.path.insert(0, "/opt/trn_rl_repo")

import numpy as np

import concourse.bacc as bacc
import concourse.mybir as mybir
from concourse.tile import TileContext
from concourse import bass_utils

IN_C, HID_C, OUT_C = 6, 128, 6
B, H, W = 8, 256, 256
WP = W + 2  # padded width
R = 32  # output rows per strip
N_CORES = 8

f32 = mybir.dt.float32
f32r = mybir.dt.float32r
Relu = mybir.ActivationFunctionType.Relu
Ident = mybir.ActivationFunctionType.Identity

# tap order matches reference: 0=center, 1=up(x[h-1]), 2=down(x[h+1]),
# 3=left(x[w-1]), 4=right(x[w+1])


def _build(repeat=1):
    nc = bacc.Bacc("TRN2", target_bir_lowering=False)

    x_d = nc.dram_tensor("x", [IN_C, H, W], f32, kind="ExternalInput")
    w1_d = nc.dram_tensor("w1p", [5 * IN_C, HID_C], f32, kind="ExternalInput")
    w2_d = nc.dram_tensor("w2p", [HID_C, 5, HID_C], f32, kind="ExternalInput")
    w3_d = nc.dram_tensor("w3p", [HID_C, 5, HID_C], f32, kind="ExternalInput")
    w4_d = nc.dram_tensor("w4p", [HID_C, 5, OUT_C], f32, kind="ExternalInput")
    b1_d = nc.dram_tensor("b1", [HID_C], f32, kind="ExternalInput")
    b2_d = nc.dram_tensor("b2", [HID_C], f32, kind="ExternalInput")
    b3_d = nc.dram_tensor("b3", [HID_C], f32, kind="ExternalInput")
    b4_d = nc.dram_tensor("b4", [OUT_C], f32, kind="ExternalInput")
    y_d = nc.dram_tensor("y", [OUT_C, H, W], f32, kind="ExternalOutput")

    with TileContext(nc) as tc:
        with (
            tc.tile_pool(name="const", bufs=1) as cpool,
            tc.tile_pool(name="bufs", bufs=1) as bpool,
            tc.tile_pool(name="io", bufs=4) as iopool,
            tc.tile_pool(name="psmain", bufs=6, space="PSUM") as pmain,
            tc.tile_pool(name="ps4", bufs=2, space="PSUM") as p4,
        ):
            # --- weights / biases (resident) ---
            w1_sb = cpool.tile([5 * IN_C, HID_C], f32r)
            nc.sync.dma_start(out=w1_sb, in_=w1_d[:, :].bitcast(f32r))
            w2_sb = cpool.tile([HID_C, 5, HID_C], f32r)
            nc.sync.dma_start(out=w2_sb, in_=w2_d[:, :, :].bitcast(f32r))
            w3_sb = cpool.tile([HID_C, 5, HID_C], f32r)
            nc.sync.dma_start(out=w3_sb, in_=w3_d[:, :, :].bitcast(f32r))
            w4_sb = cpool.tile([HID_C, 5, OUT_C], f32r)
            nc.sync.dma_start(out=w4_sb, in_=w4_d[:, :, :].bitcast(f32r))
            b1_sb = cpool.tile([HID_C, 1], f32)
            nc.sync.dma_start(out=b1_sb, in_=b1_d[:, None])
            b2_sb = cpool.tile([HID_C, 1], f32)
            nc.sync.dma_start(out=b2_sb, in_=b2_d[:, None])
            b3_sb = cpool.tile([HID_C, 1], f32)
            nc.sync.dma_start(out=b3_sb, in_=b3_d[:, None])
            b4_sb = cpool.tile([OUT_C, 1], f32)
            nc.sync.dma_start(out=b4_sb, in_=b4_d[:, None])

            # --- persistent strip buffers (bufs=1; zero-initialized once) ---
            # x30: 5 tap-groups x 6ch, pre-shifted by DMA placement.
            # group g partitions [6g,6g+6); center x(h,w) -> (slot h-a+5, col w+1)
            x30 = bpool.tile([5 * IN_C, R + 10, WP], f32r)
            h1 = bpool.tile([HID_C, R + 6, WP], f32r)  # L1 rows [a-3,b+3)
            h2 = bpool.tile([HID_C, R + 4, WP], f32r)  # L2 rows [a-2,b+2)
            h3 = bpool.tile([HID_C, R + 2, WP], f32r)  # L3 rows [a-1,b+1)

            nc.vector.memset(x30[:, :, :].bitcast(f32), 0.0)
            nc.vector.memset(h1[:, :, :].bitcast(f32), 0.0)
            nc.vector.memset(h2[:, :, :].bitcast(f32), 0.0)
            nc.vector.memset(h3[:, :, :].bitcast(f32), 0.0)

            def conv_chunk(ps, w_sb, src, s, n, first_s, last_s):
                """5 accumulating matmuls: output chunk of n rows whose
                center is src[:, s:s+n, 1:257]. first_s/last_s clamp guards
                are handled by the caller via buffer zero-slots."""
                nc.tensor.matmul(
                    ps, w_sb[:, 0, :], src[:, s : s + n, 1 : 1 + W],
                    start=True, stop=False,
                )
                nc.tensor.matmul(
                    ps, w_sb[:, 1, :], src[:, s - 1 : s - 1 + n, 1 : 1 + W],
                    start=False, stop=False,
                )
                nc.tensor.matmul(
                    ps, w_sb[:, 2, :], src[:, s + 1 : s + 1 + n, 1 : 1 + W],
                    start=False, stop=False,
                )
                nc.tensor.matmul(
                    ps, w_sb[:, 3, :], src[:, s : s + n, 0:W],
                    start=False, stop=False,
                )
                nc.tensor.matmul(
                    ps, w_sb[:, 4, :], src[:, s : s + n, 2 : 2 + W],
                    start=False, stop=True,
                )

            for rep in range(repeat):
             for a in range(0, H, R):
                b = a + R
                last = b == H
                lo_x, hi_x = max(0, a - 4), min(H, b + 4)

                if last:
                    # re-zero stale tail slots (bufs=1 reuse) before writes
                    nc.vector.memset(x30[:, hi_x - a + 4 : R + 10, :].bitcast(f32), 0.0)
                    nc.vector.memset(h1[:, 256 - (a - 3) : R + 6, :].bitcast(f32), 0.0)
                    nc.vector.memset(h2[:, 256 - (a - 2) : R + 4, :].bitcast(f32), 0.0)
                    nc.vector.memset(h3[:, 256 - (a - 1) : R + 2, :].bitcast(f32), 0.0)

                # --- load x strip, 5 shifted placements ---
                src = x_d[:, lo_x:hi_x, :].bitcast(f32r)
                o = lo_x - a
                nc.sync.dma_start(out=x30[0:6, o + 5 : hi_x - a + 5, 1 : 1 + W], in_=src)
                nc.sync.dma_start(out=x30[6:12, o + 6 : hi_x - a + 6, 1 : 1 + W], in_=src)
                nc.sync.dma_start(out=x30[12:18, o + 4 : hi_x - a + 4, 1 : 1 + W], in_=src)
                nc.sync.dma_start(out=x30[18:24, o + 5 : hi_x - a + 5, 2 : 2 + W], in_=src)
                nc.sync.dma_start(out=x30[24:30, o + 5 : hi_x - a + 5, 0:W], in_=src)

                # --- L1: rows [a-3, b+3) -> h1 (single K=30 matmul/chunk) ---
                rr = max(0, a - 3)
                hi = min(H, b + 3)
                while rr < hi:
                    n = min(2, hi - rr)
                    s = rr - a + 5
                    ps = pmain.tile([HID_C, n, W], f32, tag="ps")
                    nc.tensor.matmul(
                        ps, w1_sb[:, :], x30[:, s : s + n, 1 : 1 + W],
                        start=True, stop=True,
                    )
                    d = rr - (a - 3)
                    nc.scalar.activation(
                        h1[:, d : d + n, 1 : 1 + W], ps, Relu, bias=b1_sb
                    )
                    rr += n

                # --- L2: rows [a-2, b+2), reads h1 ---
                rr = max(0, a - 2)
                hi = min(H, b + 2)
                while rr < hi:
                    n = min(2, hi - rr)
                    s = rr - a + 3  # h1 slot of center
                    ps = pmain.tile([HID_C, n, W], f32, tag="ps")
                    conv_chunk(ps, w2_sb, h1, s, n, None, None)
                    d = rr - (a - 2)
                    nc.scalar.activation(
                        h2[:, d : d + n, 1 : 1 + W], ps, Relu, bias=b2_sb
                    )
                    rr += n

                # --- L3: rows [a-1, b+1), reads h2 ---
                rr = max(0, a - 1)
                hi = min(H, b + 1)
                while rr < hi:
                    n = min(2, hi - rr)
                    s = rr - a + 2  # h2 slot of center
                    ps = pmain.tile([HID_C, n, W], f32, tag="ps")
                    conv_chunk(ps, w3_sb, h2, s, n, None, None)
                    d = rr - (a - 1)
                    nc.scalar.activation(
                        h3[:, d : d + n, 1 : 1 + W], ps, Relu, bias=b3_sb
                    )
                    rr += n

                # --- L4: rows [a, b), reads h3, no relu ---
                rr = a
                while rr < b:
                    n = min(2, b - rr)
                    s = rr - a + 1  # h3 slot of center
                    ps = p4.tile([OUT_C, n, W], f32, tag="ps4")
                    conv_chunk(ps, w4_sb, h3, s, n, None, None)
                    yt = iopool.tile([OUT_C, n, W], f32, tag="yt")
                    nc.scalar.activation(yt, ps, Ident, bias=b4_sb)
                    nc.sync.dma_start(out=y_d[:, rr : rr + n, :], in_=yt)
                    rr += n

    nc.finalize()
    return nc


_NC_CACHE = {}


def kernel(x, w1, b1, w2, b2, w3, b3, w4, b4):
    x = np.ascontiguousarray(np.asarray(x, dtype=np.float32))
    w1 = np.asarray(w1, dtype=np.float32)
    w2 = np.asarray(w2, dtype=np.float32)
    w3 = np.asarray(w3, dtype=np.float32)
    w4 = np.asarray(w4, dtype=np.float32)

    # pack weights: lhsT layouts
    # w1p[t*6+ic, oc] = w1[oc, ic, t]
    w1p = np.ascontiguousarray(w1.transpose(2, 1, 0).reshape(5 * IN_C, HID_C))
    # w2p[ic, t, oc] = w2[oc, ic, t]
    w2p = np.ascontiguousarray(w2.transpose(1, 2, 0))
    w3p = np.ascontiguousarray(w3.transpose(1, 2, 0))
    w4p = np.ascontiguousarray(w4.transpose(1, 2, 0))

    if "nc" not in _NC_CACHE:
        _NC_CACHE["nc"] = _build()
    nc = _NC_CACHE["nc"]

    common = {
        "w1p": w1p, "w2p": w2p, "w3p": w3p, "w4p": w4p,
        "b1": np.asarray(b1, np.float32), "b2": np.asarray(b2, np.float32),
        "b3": np.asarray(b3, np.float32), "b4": np.asarray(b4, np.float32),
    }
    in_maps = [dict(common, x=x[i]) for i in range(N_CORES)]
    res = bass_utils.run_bass_kernel_spmd(nc, in_maps, core_ids=list(range(N_CORES)))
    out = np.stack([res.results[i]["y"] for i in range(N_CORES)], axis=0)
    return out
